# revision 15
# baseline (speedup 1.0000x reference)
"""Trainium2 Bass kernel for nn_GAT_1580547974673 (2-layer GAT + pair scoring).

Self-contained: hardcodes all shapes/sharding. Strategy: row-shard the NxN
attention over 8 cores (384 rows each, all 8 heads), pair scoring sharded
over P.

v2 restructure vs v1 (342us -> ...):
  - f2 gathered EARLY via its own tiny AllGather (fp32), so the DVE/ACT
    z/exp pipeline never waits on the big h gather; h AllGather overlaps
    with the first attention chunks.
  - bf16 on all high-volume paths: mask tiles, gathered h/h2/x_out,
    exp outputs (et), pair maps, h-compute matmuls. z stays fp32 (softmax
    rows are peaked; quantizing z on dominant weights doesn't average out).
  - p1/p2 prefetched at kernel start (hidden under attention).
  - single grouped DMAs for gathered tensors (f2/h2/xo) instead of 24 small.

Math restructuring (validated vs reference in numpy, bf16-emulated ~5e-3):
  - f1 = x @ (W @ a1), f2 = x @ (W @ a2)         (weight folding)
  - att_unnorm = exp(lrelu(f1_i + f2_j + M_ij)), M = 0 / -1e9 (mask pre-fold;
    exp of masked entries underflows to exactly 0)
  - no max-subtraction (|z| <= ~50, exp stays in fp32/bf16 range)
  - rowsum via ones-augmented h in the att @ [h|1] matmul; divide after
  - elu(t) = relu(t) + min(exp(t), 1) - 1        (single Exp, fused combine)
Layout: attention computed transposed (j on partitions, i on free dim) so the
contraction dim of att @ h lands on partitions; per-partition scalars are f2,
free-dim broadcast of f1 built once per head via gpsimd partition_broadcast.
"""
import numpy as np
from contextlib import ExitStack

import concourse.bass as bass
import concourse.bacc as bacc
import concourse.mybir as mybir
import concourse.tile as tile
import concourse.dve_ops as dve_ops
from concourse.dve_ops import DveOp
from concourse.dve_spec import Spec, Src0, Src1, One, maxx, minn, relu, lower
from concourse.dve_uop import DveOpSpec
from concourse.bass_utils import run_bass_kernel_spmd
from concourse.masks import make_identity

F32 = mybir.dt.float32
BF16 = mybir.dt.bfloat16
AF = mybir.ActivationFunctionType

# problem shapes (hardcoded per spec)
N, FIN, FH, H, NPAIR = 3072, 512, 64, 8, 2048
NC = 8
IB = N // NC            # 384 rows per core
PB = NPAIR // NC        # 256 pairs per core
NJ = N // 128           # 24 j-blocks
KB = FIN // 128         # 4 k-blocks of the feature dim
SUB = IB // 128         # 3 sub-blocks of the core's row slice
CH = 8                  # j-blocks per exp chunk
NCH = NJ // CH
MASKVAL = -1.0e9
ALPHA = 0.2
HC = FH + 1             # per-head gathered columns: h | ones
HCOLS = H * HC          # 520

SIM_NOCOLL = False  # replace collectives with local DMA (for TimelineSim)


def _register_ops():
    """Register the two custom DVE ops (idempotent)."""
    from concourse.dve_spec import C0, C1
    defs = []
    if "GAT_MASK_LRELU" not in dve_ops._SUB_OPCODE_FOR_NAME:
        s = (Src0 + Src1) + C0
        defs.append(DveOp(
            "GAT_MASK_LRELU",
            Spec(body=maxx(s, s * C1),
                 reference=lambda in0, in1, s0, s1, imm2: np.maximum(
                     (in0 + in1) + s0, ((in0 + in1) + s0) * s1)),
            subdim=False, uops_sha={}))
    if "GAT_ELU_COMBINE" not in dve_ops._SUB_OPCODE_FOR_NAME:
        # out = relu(t) + min(E, 1) - 1  with t=Src0, E=Src1(=exp(t))
        defs.append(DveOp(
            "GAT_ELU_COMBINE",
            Spec(body=relu(Src0) + minn(Src1, One) - One,
                 reference=lambda in0, in1, s0, s1, imm2:
                     np.maximum(in0, 0) + np.minimum(in1, 1.0) - 1.0),
            subdim=False, uops_sha={}))
    for op in defs:
        for ver in ("v3", "v4"):
            tmp = DveOpSpec(name=op.name, opcode=0,
                            uops=lower(op.spec, ver=ver), rd1_en=True)
            op.uops_sha[ver] = tmp.sha(ver)
        dve_ops.OPS.append(op)
        dve_ops.CUSTOM_DVE_SPECS[op.name] = op.spec
        dve_ops._SUB_OPCODE_FOR_NAME[op.name] = (
            dve_ops._CUSTOM_DVE_ROW_BASE + len(dve_ops.OPS) - 1)
    ops = {op.name: op for op in dve_ops.OPS}
    return ops["GAT_MASK_LRELU"], ops["GAT_ELU_COMBINE"]


def build(nc, reps=1):
    op_mask_lrelu, op_elu = _register_ops()

    # ---- I/O ----
    xTs_in = nc.dram_tensor("xTs_in", [FIN, IB], F32, kind="ExternalInput")
    xTsb_in = nc.dram_tensor("xTsb_in", [FIN, IB], BF16, kind="ExternalInput")
    maskT_in = nc.dram_tensor("maskT_in", [N, IB], BF16, kind="ExternalInput")
    Wall_in = nc.dram_tensor("Wall_in", [FIN, FIN], BF16, kind="ExternalInput")
    W12_in = nc.dram_tensor("W12_in", [FIN, 2 * H], F32, kind="ExternalInput")
    Wo_in = nc.dram_tensor("Wo_in", [FIN, FH + 2], F32, kind="ExternalInput")
    wgt_in = nc.dram_tensor("wgt_in", [FH, FH], F32, kind="ExternalInput")
    p1T_in = nc.dram_tensor("p1T_in", [N, PB], BF16, kind="ExternalInput")
    p2T_in = nc.dram_tensor("p2T_in", [N, PB], BF16, kind="ExternalInput")
    scores_out = nc.dram_tensor("scores_out", [1, PB], F32, kind="ExternalOutput")

    groups = [list(range(NC))]

    with tile.TileContext(nc) as tc, ExitStack() as octx:
      for rep in range(reps):
        R = f"_r{rep}"
        ctx = ExitStack()
        octx.enter_context(ctx)
        tiny = ctx.enter_context(tc.tile_pool(name="tiny" + R, bufs=1))
        xcp = ctx.enter_context(tc.tile_pool(name="xcp" + R, bufs=1))
        h2pool = ctx.enter_context(tc.tile_pool(name="h2pool" + R, bufs=1))
        npool = ctx.enter_context(tc.tile_pool(name="npool" + R, bufs=2))
        ppool = ctx.enter_context(tc.tile_pool(name="ppool" + R, bufs=1))
        dram = ctx.enter_context(tc.tile_pool(name="dram" + R, bufs=1, space="DRAM"))
        ps_small = ctx.enter_context(tc.tile_pool(name="ps_small" + R, bufs=2, space="PSUM"))
        ps_h_pool = ctx.enter_context(tc.tile_pool(name="ps_h" + R, bufs=2, space="PSUM"))
        ps_hp_pool = ctx.enter_context(tc.tile_pool(name="ps_hp" + R, bufs=4, space="PSUM"))
        ctx_att1 = ctx.enter_context(ExitStack())
        maskp = ctx_att1.enter_context(tc.tile_pool(name="maskp" + R, bufs=1))
        ztp = ctx_att1.enter_context(tc.tile_pool(name="ztp" + R, bufs=2))
        ep = ctx_att1.enter_context(tc.tile_pool(name="ep" + R, bufs=4))
        ctx_prep = ctx.enter_context(ExitStack())
        fpool = ctx_prep.enter_context(tc.tile_pool(name="fpool" + R, bufs=1))
        hpool = ctx_prep.enter_context(tc.tile_pool(name="hpool" + R, bufs=1))
        ctx_bc = ctx.enter_context(ExitStack())
        cst = ctx_bc.enter_context(tc.tile_pool(name="cst" + R, bufs=1))

        # ---- constant loads ----
        # Two DGE rings: nc.sync (HWDGE/SP) carries the latency-critical small
        # transfers in need-order; nc.gpsimd (SWDGE) carries bulk prefetches so
        # they can't FIFO-block the critical path.
        xTs = []
        xTsb = []
        Wall = []
        W12 = []
        Wo = []
        xTs_g = cst.tile([128, KB, IB], F32, name="xTs_g")
        nc.sync.dma_start(xTs_g[:], xTs_in[:].rearrange("(k p) c -> p k c", p=128))
        xTs = [xTs_g[:, kb, :] for kb in range(KB)]
        W12_g = cst.tile([128, KB, 2 * H], F32, name="W12_g")
        nc.sync.dma_start(W12_g[:], W12_in[:].rearrange("(k p) c -> p k c", p=128))
        W12 = [W12_g[:, kb, :] for kb in range(KB)]
        xTsb_g = cst.tile([128, KB, IB], BF16, name="xTsb_g")
        nc.gpsimd.dma_start(xTsb_g[:], xTsb_in[:].rearrange("(k p) c -> p k c", p=128))
        xTsb = [xTsb_g[:, kb, :] for kb in range(KB)]
        Wall_g = cst.tile([128, KB, FIN], BF16, name="Wall_g")
        nc.gpsimd.dma_start(Wall_g[:], Wall_in[:].rearrange("(k p) c -> p k c", p=128))
        Wall = [Wall_g[:, kb, :] for kb in range(KB)]
        # mask tiles (stay resident through both attention layers); first two
        # groups on the fast ring, the rest later (consumed mid-attention)
        GBM = 4
        mback = []
        for g4 in range(NJ // GBM):
            m = maskp.tile([128, GBM, IB], BF16, name=f"maskT{g4}")
            if g4 < 2:
                nc.sync.dma_start(
                    m[:], maskT_in[g4 * GBM * 128:(g4 + 1) * GBM * 128, :]
                    .rearrange("(g p) c -> p g c", p=128))
            mback.append(m)
        maskT = [mback[jb // GBM][:, jb % GBM, :] for jb in range(NJ)]
        ones8 = tiny.tile([128, H], BF16)
        nc.gpsimd.memset(ones8[:], 1.0)
        ones64 = tiny.tile([FH, 1], F32)
        nc.gpsimd.memset(ones64[:], 1.0)
        ident = tiny.tile([128, 128], F32)
        make_identity(nc, ident[:])

        # ---- stage B: f-pass (fp32, exact): F1 (free-dim) + F2 (natural) ----
        ps_ft = ps_small.tile([2 * H, IB], F32, tag="pss")
        for kb in range(KB):
            nc.tensor.matmul(ps_ft[:], W12[kb][:], xTs[kb][:],
                             start=(kb == 0), stop=(kb == KB - 1))
        FTsb = fpool.tile([2 * H, IB], F32)
        nc.scalar.copy(FTsb[:], ps_ft[:])
        ft_d = dram.tile([2 * H, IB], F32)
        nc.sync.dma_start(ft_d[:], FTsb[:])

        F2loc_sb = []
        for s in range(SUB):
            ps_f2 = ps_small.tile([128, H], F32, tag="pss")
            for kb in range(KB):
                nc.tensor.matmul(ps_f2[:], xTs[kb][:, s * 128:(s + 1) * 128],
                                 W12[kb][:, H:2 * H],
                                 start=(kb == 0), stop=(kb == KB - 1))
            t = fpool.tile([128, H], F32, name=f"F2loc{s}")
            nc.scalar.copy(t[:], ps_f2[:])
            F2loc_sb.append(t)

        # early tiny AllGather of f2 (fp32) so z/exp never waits on the h AG
        f2loc_d = dram.tile([IB, H], F32)
        f2g_d = dram.tile([N, H], F32, addr_space="Shared")
        for s in range(SUB):
            nc.sync.dma_start(f2loc_d[s * 128:(s + 1) * 128, :], F2loc_sb[s][:])
        if SIM_NOCOLL:
            nc.sync.dma_start(f2g_d[0:IB, :], f2loc_d[:])
        else:
            nc.gpsimd.collective_compute(
                "AllGather", mybir.AluOpType.bypass, replica_groups=groups,
                ins=[f2loc_d[:].opt()], outs=[f2g_d[:].opt()])
        f2sb = fpool.tile([128, NJ, H], F32)
        nc.sync.dma_start(
            f2sb[:], f2g_d[:].rearrange("(g p) c -> p g c", p=128))

        # f1 broadcast tiles per head (row bounced to partition 0 via DRAM)
        f1b = []
        for h in range(H):
            row = fpool.tile([1, IB], F32, name=f"f1row{h}")
            nc.sync.dma_start(row[:], ft_d[h:h + 1, :])
            t = fpool.tile([128, IB], F32, name=f"f1b{h}")
            nc.gpsimd.partition_broadcast(t[:], row[:])
            f1b.append(t)

        # remaining mask groups (needed from ~chunk 2 onward)
        for g4 in range(2, NJ // GBM):
            nc.sync.dma_start(
                mback[g4][:], maskT_in[g4 * GBM * 128:(g4 + 1) * GBM * 128, :]
                .rearrange("(g p) c -> p g c", p=128))

        # ---- stage C: local h (bf16) -> haug layout -> gather ----
        hloc_d = dram.tile([IB, HCOLS], BF16)
        for s in range(SUB):
            ps_h = ps_h_pool.tile([128, FIN], F32, tag="ph")
            for kb in range(KB):
                nc.tensor.matmul(ps_h[:], xTsb[kb][:, s * 128:(s + 1) * 128],
                                 Wall[kb][:],
                                 start=(kb == 0), stop=(kb == KB - 1))
            hsb = fpool.tile([128, HCOLS], BF16, name=f"hloc{s}")
            hsb3 = hsb[:].rearrange("p (h f) -> p h f", h=H)
            nc.scalar.copy(hsb3[:, :, 0:FH],
                           ps_h[:].rearrange("p (h f) -> p h f", h=H))
            nc.scalar.copy(hsb3[:, :, FH], ones8[:])
            nc.gpsimd.dma_start(hloc_d[s * 128:(s + 1) * 128, :], hsb[:])

        hg_d = dram.tile([N, HCOLS], BF16, addr_space="Shared")
        if SIM_NOCOLL:
            nc.gpsimd.dma_start(hg_d[0:IB, :], hloc_d[:])
        else:
            nc.gpsimd.collective_compute(
                "AllGather", mybir.AluOpType.bypass, replica_groups=groups,
                ins=[hloc_d[:].opt()], outs=[hg_d[:].opt()])

        # DMA gathered tensor back, 8 j-blocks per transfer (SWDGE ring; the
        # first attention chunks only need DVE/ACT inputs, not haug)
        GB = 8
        hback = []
        for g4 in range(NJ // GB):
            ht = hpool.tile([128, GB, HCOLS], BF16, name=f"hback{g4}")
            nc.gpsimd.dma_start(
                ht[:], hg_d[g4 * GB * 128:(g4 + 1) * GB * 128, :].rearrange(
                    "(g p) c -> p g c", p=128))
            hback.append(ht)
        haug_r = [hback[jb // GB][:, jb % GB, :] for jb in range(NJ)]

        ctx_bc.close()

        # ---- stage D: layer-1 attention, per head ----
        # per-head output tiles on partitions 0..63 so the elu op writes them
        # directly (no partition-moving DMA); layer-2 matmuls contract per-head
        xcT8 = [xcp.tile([FH, IB], F32, name=f"xcT8_{h}") for h in range(H)]

        def attention(head, f2col_of, f1b_t, haug_of, out_t):
            """One attention unit: out_t[:] = elu-ish(att @ h)."""
            ps_hp = ps_hp_pool.tile([FH + 1, IB], F32, tag="hp",
                                    name=f"ps_hp{head}")
            for c in range(NCH):
                zt = ztp.tile([128, CH, IB], F32, tag="zt", name=f"zt{head}_{c}")
                for g in range(CH):
                    jb = c * CH + g
                    nc.vector._custom_dve(
                        op_mask_lrelu, out=zt[:, g, :], in0=f1b_t[:],
                        in1=maskT[jb], s0=f2col_of(jb), s1=ALPHA)
                et = ep.tile([128, CH, IB], BF16, tag="et", name=f"et{head}_{c}")
                nc.scalar.activation(et[:], zt[:], AF.Exp)
                for g in range(CH):
                    jb = c * CH + g
                    nc.tensor.matmul(
                        ps_hp[:], haug_of(jb),
                        et[:, g, :], start=(jb == 0), stop=(jb == NJ - 1))
            # normalize + elu
            # rowsum lives at PSUM partition FH; copy within-partition to
            # SBUF, then DMA (the only partition-moving engine) to partition 0
            rs64 = npool.tile([128, IB], F32, bufs=1, tag="rs64", name=f"rs64_{head}")
            nc.scalar.copy(rs64[FH:FH + 1, :], ps_hp[FH:FH + 1, :])
            rsum = npool.tile([1, IB], F32, bufs=1, tag="rsum", name=f"rsum{head}")
            nc.sync.dma_start(rsum[:], rs64[FH:FH + 1, :])
            rrow = npool.tile([1, IB], F32, bufs=1, tag="rrow", name=f"rrow{head}")
            nc.vector.reciprocal_approx_fast(rrow[:], rsum[:])
            rb = npool.tile([FH, IB], F32, tag="rb", name=f"rb{head}")
            nc.gpsimd.partition_broadcast(rb[:], rrow[:])
            hp_sb = npool.tile([FH, IB], F32, tag="hps", name=f"hps{head}")
            nc.scalar.copy(hp_sb[:], ps_hp[0:FH, :])
            t_n = npool.tile([FH, IB], F32, tag="tn", name=f"tn{head}")
            nc.gpsimd.tensor_mul(t_n[:], hp_sb[:], rb[:])
            e_n = npool.tile([FH, IB], F32, tag="en", name=f"en{head}")
            nc.scalar.activation(e_n[:], t_n[:], AF.Exp)
            nc.vector._custom_dve(op_elu, out=out_t[:], in0=t_n[:], in1=e_n[:])

        p1g = []
        p2g = []
        GP = 8
        for head in range(H):
            attention(head,
                      lambda jb, h=head: f2sb[:, jb, h:h + 1],
                      f1b[head],
                      lambda jb, h=head: haug_r[jb][:, h * HC:h * HC + HC],
                      xcT8[head])
            if head == 0:
                # bulk prefetch for the tail stages, issued once the attention
                # pipeline is running (DMA + Pool are idle from here on)
                Wo_h = tiny.tile([64, H, FH + 2], F32, name="Wo_h")
                nc.gpsimd.dma_start(
                    Wo_h[:], Wo_in[:].rearrange("(h q) c -> q h c", q=64))
                wgt = tiny.tile([FH, FH], F32)
                nc.gpsimd.dma_start(wgt[:], wgt_in[:])
            elif head in (1, 2) and not p2g:
                for g4 in range(NJ // GP):
                    tp1 = ppool.tile([128, GP, PB], BF16, name=f"p1_{g4}")
                    nc.gpsimd.dma_start(
                        tp1[:], p1T_in[g4 * GP * 128:(g4 + 1) * GP * 128, :]
                        .rearrange("(g p) c -> p g c", p=128))
                    p1g.append(tp1)
                    tp2 = ppool.tile([128, GP, PB], BF16, name=f"p2_{g4}")
                    nc.gpsimd.dma_start(
                        tp2[:], p2T_in[g4 * GP * 128:(g4 + 1) * GP * 128, :]
                        .rearrange("(g p) c -> p g c", p=128))
                    p2g.append(tp2)
        p1r = [p1g[jb // GP][:, jb % GP, :] for jb in range(NJ)]
        p2r = [p2g[jb // GP][:, jb % GP, :] for jb in range(NJ)]

        ctx_prep.close()
        xopool = ctx.enter_context(tc.tile_pool(name="xopool" + R, bufs=1))
        epool = ctx.enter_context(tc.tile_pool(name="epool" + R, bufs=1))

        # ---- stage E: layer-2 h (local rows, fp32 in / bf16 out) -> gather ----
        h2loc_d = dram.tile([IB, FH + 2], BF16)
        for s in range(SUB):
            ps_h2 = ps_small.tile([128, FH + 1], F32, tag="pss")
            for h in range(H):
                nc.tensor.matmul(ps_h2[:], xcT8[h][:, s * 128:(s + 1) * 128],
                                 Wo_h[:, h, 0:FH + 1],
                                 start=(h == 0), stop=(h == H - 1))
            t = h2pool.tile([128, FH + 2], BF16, name=f"h2loc{s}")
            nc.scalar.copy(t[:, 0:FH], ps_h2[:, 0:FH])
            nc.scalar.copy(t[:, FH:FH + 1], ones8[:, 0:1])
            nc.scalar.copy(t[:, FH + 1:FH + 2], ps_h2[:, FH:FH + 1])
            nc.sync.dma_start(h2loc_d[s * 128:(s + 1) * 128, :], t[:])
        # f1_2 as a free-dim row
        ps_f12 = ps_small.tile([1, IB], F32, tag="pss")
        for h in range(H):
            nc.tensor.matmul(ps_f12[:], Wo_h[:, h, FH + 1:FH + 2], xcT8[h][:],
                             start=(h == 0), stop=(h == H - 1))
        f12row = h2pool.tile([1, IB], F32)
        nc.scalar.copy(f12row[:], ps_f12[:])
        f12b = h2pool.tile([128, IB], F32)
        nc.gpsimd.partition_broadcast(f12b[:], f12row[:])

        h2g_d = dram.tile([N, FH + 2], BF16, addr_space="Shared")
        if SIM_NOCOLL:
            nc.sync.dma_start(h2g_d[0:IB, :], h2loc_d[:])
        else:
            nc.gpsimd.collective_compute(
                "AllGather", mybir.AluOpType.bypass, replica_groups=groups,
                ins=[h2loc_d[:].opt()], outs=[h2g_d[:].opt()])

        h2back = h2pool.tile([128, NJ, FH + 2], BF16)
        nc.sync.dma_start(
            h2back[:], h2g_d[:].rearrange("(g p) c -> p g c", p=128))
        h2r = [h2back[:, jb, 0:FH + 1] for jb in range(NJ)]
        # layer-2 f2 column cast to fp32 (per-partition scalar for the z op)
        f22sb = h2pool.tile([128, NJ], F32)
        nc.scalar.copy(f22sb[:], h2back[:, :, FH + 1])

        # ---- stage F: layer-2 attention (single head) ----
        xoT = h2pool.tile([FH, IB], F32)

        ps_hp2 = ps_hp_pool.tile([FH + 1, IB], F32, tag="hp", name="ps_hp2")
        for c in range(NCH):
            zt = ztp.tile([128, CH, IB], F32, tag="zt", name=f"zt2_{c}")
            for g in range(CH):
                jb = c * CH + g
                nc.vector._custom_dve(
                    op_mask_lrelu, out=zt[:, g, :], in0=f12b[:],
                    in1=maskT[jb], s0=f22sb[:, jb:jb + 1], s1=ALPHA)
            et = ep.tile([128, CH, IB], BF16, tag="et", name=f"et2_{c}")
            nc.scalar.activation(et[:], zt[:], AF.Exp)
            for g in range(CH):
                jb = c * CH + g
                nc.tensor.matmul(ps_hp2[:], h2r[jb], et[:, g, :],
                                 start=(jb == 0), stop=(jb == NJ - 1))
        rs64b = npool.tile([128, IB], F32, bufs=1, tag="rs64", name="rs64b")
        nc.scalar.copy(rs64b[FH:FH + 1, :], ps_hp2[FH:FH + 1, :])
        rsum2 = npool.tile([1, IB], F32, bufs=1, tag="rsum", name="rsum2")
        nc.sync.dma_start(rsum2[:], rs64b[FH:FH + 1, :])
        rrow2 = npool.tile([1, IB], F32, bufs=1, tag="rrow", name="rrow2")
        nc.vector.reciprocal_approx_fast(rrow2[:], rsum2[:])
        rb2 = npool.tile([FH, IB], F32, tag="rb", name="rb2")
        nc.gpsimd.partition_broadcast(rb2[:], rrow2[:])
        hp_sb2 = npool.tile([FH, IB], F32, tag="hps", name="hps2")
        nc.scalar.copy(hp_sb2[:], ps_hp2[0:FH, :])
        t_n2 = npool.tile([FH, IB], F32, tag="tn", name="tn2")
        nc.gpsimd.tensor_mul(t_n2[:], hp_sb2[:], rb2[:])
        e_n2 = npool.tile([FH, IB], F32, tag="en", name="en2")
        nc.scalar.activation(e_n2[:], t_n2[:], AF.Exp)
        nc.vector._custom_dve(op_elu, out=xoT[:], in0=t_n2[:], in1=e_n2[:])

        # ---- stage G: x_out natural layout (bf16) + gather ----
        xoloc_d = dram.tile([IB, FH], BF16)
        for s in range(SUB):
            ps_tr = ps_small.tile([128, FH], F32, tag="pss")
            nc.tensor.transpose(ps_tr[:], xoT[:, s * 128:(s + 1) * 128], ident[0:FH, 0:FH])
            t = xopool.tile([128, FH], BF16, name=f"xol{s}")
            nc.scalar.copy(t[:], ps_tr[:])
            nc.sync.dma_start(xoloc_d[s * 128:(s + 1) * 128, :], t[:])
        xog_d = dram.tile([N, FH], BF16, addr_space="Shared")
        if SIM_NOCOLL:
            nc.sync.dma_start(xog_d[0:IB, :], xoloc_d[:])
        else:
            nc.gpsimd.collective_compute(
                "AllGather", mybir.AluOpType.bypass, replica_groups=groups,
                ins=[xoloc_d[:].opt()], outs=[xog_d[:].opt()])

        xorg = xopool.tile([128, NJ, FH], BF16)
        nc.sync.dma_start(
            xorg[:], xog_d[:].rearrange("(g p) c -> p g c", p=128))
        xor_ = [xorg[:, jb, :] for jb in range(NJ)]

        # ---- stage H: pair embeddings + scores ----
        ps_e1 = ps_small.tile([FH, PB], F32, tag="pss", name="ps_e1")
        for jb in range(NJ):
            nc.tensor.matmul(ps_e1[:], xor_[jb], p1r[jb],
                             start=(jb == 0), stop=(jb == NJ - 1))
        e1sb = epool.tile([FH, PB], F32)
        nc.scalar.copy(e1sb[:], ps_e1[:])
        ps_e2 = ps_small.tile([FH, PB], F32, tag="pss", name="ps_e2")
        for jb in range(NJ):
            nc.tensor.matmul(ps_e2[:], xor_[jb], p2r[jb],
                             start=(jb == 0), stop=(jb == NJ - 1))
        e2sb = epool.tile([FH, PB], F32)
        nc.scalar.copy(e2sb[:], ps_e2[:])

        ps_g = ps_small.tile([FH, PB], F32, tag="pss", name="ps_g")
        nc.tensor.matmul(ps_g[:], wgt[:], e1sb[:], start=True, stop=True)
        prod = epool.tile([FH, PB], F32)
        nc.vector.tensor_mul(prod[:], ps_g[:], e2sb[:])
        ps_s = ps_small.tile([1, PB], F32, tag="pss", name="ps_s")
        nc.tensor.matmul(ps_s[:], ones64[:], prod[:], start=True, stop=True)
        srow = epool.tile([1, PB], F32)
        nc.scalar.copy(srow[:], ps_s[:])
        nc.sync.dma_start(scores_out[:], srow[:])
        ctx.close()

    return nc


_CACHE = {}


def _get_nc(reps=1):
    key = f"nc{reps}"
    if key not in _CACHE:
        nc = bacc.Bacc(None, target_bir_lowering=False, debug=False, num_devices=NC)
        build(nc, reps=reps)
        nc.compile()
        _CACHE[key] = nc
    return _CACHE[key]


def _bf16(a):
    import ml_dtypes
    return np.ascontiguousarray(np.asarray(a, np.float32).astype(ml_dtypes.bfloat16))


def prep_inputs(x, adj, pair1_map, pair2_map, Wh, a1h, a2h, W_out, a1_out,
                a2_out, weight):
    x = np.ascontiguousarray(np.asarray(x, np.float32))
    adj = np.asarray(adj)
    maskT = np.where(adj > 0, np.float32(0.0), np.float32(MASKVAL)).T  # [j, i]
    maskT = _bf16(maskT)
    xT = np.ascontiguousarray(x.T)                                     # [FIN, N]
    Wall = _bf16(np.ascontiguousarray(
        np.transpose(np.asarray(Wh, np.float64), (1, 0, 2)).reshape(FIN, H * FH)))
    w1 = np.einsum("hkf,hf->kh", np.asarray(Wh, np.float64), np.asarray(a1h, np.float64))
    w2 = np.einsum("hkf,hf->kh", np.asarray(Wh, np.float64), np.asarray(a2h, np.float64))
    W12 = np.concatenate([w1, w2], axis=1).astype(np.float32)          # [FIN, 16]
    w1o = np.asarray(W_out, np.float64) @ np.asarray(a1_out, np.float64)
    w2o = np.asarray(W_out, np.float64) @ np.asarray(a2_out, np.float64)
    Wo = np.concatenate([np.asarray(W_out, np.float64), w2o[:, None],
                         w1o[:, None]], axis=1).astype(np.float32)     # [FIN, 66]
    p1T = _bf16(np.asarray(pair1_map, np.float32).T)                   # [N, NPAIR]
    p2T = _bf16(np.asarray(pair2_map, np.float32).T)
    wgt = np.ascontiguousarray(np.asarray(weight, np.float32))

    in_maps = []
    for c in range(NC):
        i0, i1 = c * IB, (c + 1) * IB
        p0, p1 = c * PB, (c + 1) * PB
        xs = np.ascontiguousarray(xT[:, i0:i1])
        in_maps.append({
            "xTs_in": xs,
            "xTsb_in": _bf16(xs),
            "maskT_in": np.ascontiguousarray(maskT[:, i0:i1]),
            "Wall_in": Wall,
            "W12_in": W12,
            "Wo_in": Wo,
            "wgt_in": wgt,
            "p1T_in": np.ascontiguousarray(p1T[:, p0:p1]),
            "p2T_in": np.ascontiguousarray(p2T[:, p0:p1]),
        })
    return in_maps


def run(inputs, trace=False, **kw):
    nc = _get_nc()
    in_maps = prep_inputs(**inputs)
    res = run_bass_kernel_spmd(nc, in_maps, list(range(NC)), trace=trace, **kw)
    scores = np.concatenate(
        [res.results[c]["scores_out"].reshape(-1) for c in range(NC)])
    return scores.astype(np.float32), res


def kernel(**inputs):
    return run(inputs)[0]


def _make_fn(nc, in_maps):
    import jax
    from jax.sharding import Mesh, PartitionSpec, NamedSharding
    from jax.experimental.shard_map import shard_map
    from concourse import bass2jax
    import concourse.mybir as _mb

    bass2jax.install_neuronx_cc_hook()
    partition_name = nc.partition_id_tensor.name if nc.partition_id_tensor else None
    in_names, out_names, out_avals, zero_outs = [], [], [], []
    for alloc in nc.m.functions[0].allocations:
        if not isinstance(alloc, _mb.MemoryLocationSet):
            continue
        name = alloc.memorylocations[0].name
        if alloc.kind == "ExternalInput":
            if name != partition_name:
                in_names.append(name)
        elif alloc.kind == "ExternalOutput":
            shape = list(alloc.tensor_shape)
            npdt = _mb.dt.np(alloc.dtype)
            out_names.append(name)
            out_avals.append(jax.core.ShapedArray(shape, npdt))
            zero_outs.append(np.zeros(shape, npdt))
    n_params = len(in_names)
    n_outs = len(out_names)
    all_in_names = list(in_names) + list(out_names)
    if partition_name is not None:
        all_in_names.append(partition_name)

    def _body(*args):
        operands = list(args)
        if partition_name is not None:
            operands.append(bass2jax.partition_id_tensor())
        outs = bass2jax._bass_exec_p.bind(
            *operands, out_avals=tuple(out_avals), in_names=tuple(all_in_names),
            out_names=tuple(out_names), lowering_input_output_aliases=(),
            sim_require_finite=True, sim_require_nnan=True, nc=nc)
        return tuple(outs)

    devices = jax.devices()[:NC]
    mesh = Mesh(np.asarray(devices), ("core",))
    in_specs = (PartitionSpec("core"),) * (n_params + n_outs)
    out_specs = (PartitionSpec("core"),) * n_outs
    fn = jax.jit(shard_map(_body, mesh=mesh, in_specs=in_specs,
                           out_specs=out_specs, check_rep=False),
                 keep_unused=True)
    concat_in = [
        np.concatenate([np.asarray(in_maps[c][nm]) for c in range(NC)], axis=0)
        for nm in in_names]
    concat_zeros = [np.zeros((NC * z.shape[0], *z.shape[1:]), z.dtype)
                    for z in zero_outs]
    sh = NamedSharding(mesh, PartitionSpec("core"))
    dev_in = [jax.device_put(a, sh) for a in concat_in]
    dev_zero = [jax.device_put(a, sh) for a in concat_zeros]
    return fn, dev_in, dev_zero


def bench(inputs, iters=6, kreps=5):
    """Device time per kernel pass, via the in-NEFF replication slope."""
    import time
    import jax
    in_maps = prep_inputs(**inputs)
    fns = {}
    for reps in (1, kreps):
        nc = _get_nc(reps=reps)
        fn, dev_in, dev_zero = _make_fn(nc, in_maps)
        jax.block_until_ready(fn(*dev_in, *dev_zero))  # warm/compile
        fns[reps] = (fn, dev_in, dev_zero)

    def once(reps):
        fn, dev_in, dev_zero = fns[reps]
        t0 = time.perf_counter()
        jax.block_until_ready(fn(*dev_in, *dev_zero))
        return time.perf_counter() - t0

    t1s, tks, diffs = [], [], []
    for _ in range(3 * iters):
        a = once(1)
        b = once(kreps)
        c = once(1)
        t1s += [a, c]
        tks.append(b)
        diffs.append(b - (a + c) / 2)
    diffs.sort()
    n = len(diffs)
    med = diffs[n // 2]
    scale = 1e9 / (kreps - 1)
    out = {
        "t1_ns": min(t1s) * 1e9,
        f"t{kreps}_ns": min(tks) * 1e9,
        "p25_ns": diffs[n // 4] * scale,
        "p75_ns": diffs[(3 * n) // 4] * scale,
        "minmin_ns": (min(tks) - min(t1s)) * scale,
        "pooled_med_ns": med * scale,
        "per_exec_ns": max(med * scale, 0.0),
    }
    return out


if __name__ == "__main__":
    rng = np.random.default_rng(0)
    ins = dict(
        x=rng.standard_normal((N, FIN), dtype=np.float32),
        adj=(rng.random((N, N)) < 0.5).astype(np.int32),
        pair1_map=rng.standard_normal((NPAIR, N), dtype=np.float32),
        pair2_map=rng.standard_normal((NPAIR, N), dtype=np.float32),
        Wh=rng.standard_normal((H, FIN, FH), dtype=np.float32) * 0.1,
        a1h=rng.standard_normal((H, FH), dtype=np.float32) * 0.3,
        a2h=rng.standard_normal((H, FH), dtype=np.float32) * 0.3,
        W_out=rng.standard_normal((FIN, FH), dtype=np.float32) * 0.1,
        a1_out=rng.standard_normal((FH,), dtype=np.float32) * 0.3,
        a2_out=rng.standard_normal((FH,), dtype=np.float32) * 0.3,
        weight=rng.standard_normal((FH, FH), dtype=np.float32) * 0.1,
    )
    out = kernel(**ins)
    print("scores:", out.shape, out[:8])


# revision 16
# speedup vs baseline: 13.3574x; 13.3574x over previous
"""Trainium2 Bass kernel for nn_GAT_1580547974673 (2-layer GAT + pair scoring).

Self-contained: hardcodes all shapes/sharding. Strategy: row-shard the NxN
attention over 8 cores (384 rows each, all 8 heads), pair scoring sharded
over P.

v2 restructure vs v1 (342us -> ...):
  - f2 gathered EARLY via its own tiny AllGather (fp32), so the DVE/ACT
    z/exp pipeline never waits on the big h gather; h AllGather overlaps
    with the first attention chunks.
  - bf16 on all high-volume paths: mask tiles, gathered h/h2/x_out,
    exp outputs (et), pair maps, h-compute matmuls. z stays fp32 (softmax
    rows are peaked; quantizing z on dominant weights doesn't average out).
  - p1/p2 prefetched at kernel start (hidden under attention).
  - single grouped DMAs for gathered tensors (f2/h2/xo) instead of 24 small.

Math restructuring (validated vs reference in numpy, bf16-emulated ~5e-3):
  - f1 = x @ (W @ a1), f2 = x @ (W @ a2)         (weight folding)
  - att_unnorm = exp(lrelu(f1_i + f2_j + M_ij)), M = 0 / -1e9 (mask pre-fold;
    exp of masked entries underflows to exactly 0)
  - no max-subtraction (|z| <= ~50, exp stays in fp32/bf16 range)
  - rowsum via ones-augmented h in the att @ [h|1] matmul; divide after
  - elu(t) = relu(t) + min(exp(t), 1) - 1        (single Exp, fused combine)
Layout: attention computed transposed (j on partitions, i on free dim) so the
contraction dim of att @ h lands on partitions; per-partition scalars are f2,
free-dim broadcast of f1 built once per head via gpsimd partition_broadcast.
"""
import numpy as np
from contextlib import ExitStack

import concourse.bass as bass
import concourse.bacc as bacc
import concourse.mybir as mybir
import concourse.tile as tile
import concourse.dve_ops as dve_ops
from concourse.dve_ops import DveOp
from concourse.dve_spec import Spec, Src0, Src1, One, maxx, minn, relu, lower
from concourse.dve_uop import DveOpSpec
from concourse.bass_utils import run_bass_kernel_spmd
from concourse.masks import make_identity

F32 = mybir.dt.float32
BF16 = mybir.dt.bfloat16
AF = mybir.ActivationFunctionType

# problem shapes (hardcoded per spec)
N, FIN, FH, H, NPAIR = 3072, 512, 64, 8, 2048
NC = 8
IB = N // NC            # 384 rows per core
PB = NPAIR // NC        # 256 pairs per core
NJ = N // 128           # 24 j-blocks
KB = FIN // 128         # 4 k-blocks of the feature dim
SUB = IB // 128         # 3 sub-blocks of the core's row slice
CH = 8                  # j-blocks per exp chunk
NCH = NJ // CH
MASKVAL = -1.0e9
ALPHA = 0.2
HC = FH + 1             # per-head gathered columns: h | ones
HCOLS = H * HC          # 520

SIM_NOCOLL = False  # replace collectives with local DMA (for TimelineSim)


def _register_ops():
    """Register the two custom DVE ops (idempotent)."""
    from concourse.dve_spec import C0, C1
    defs = []
    if "GAT_MASK_LRELU" not in dve_ops._SUB_OPCODE_FOR_NAME:
        s = (Src0 + Src1) + C0
        defs.append(DveOp(
            "GAT_MASK_LRELU",
            Spec(body=maxx(s, s * C1),
                 reference=lambda in0, in1, s0, s1, imm2: np.maximum(
                     (in0 + in1) + s0, ((in0 + in1) + s0) * s1)),
            subdim=False, uops_sha={}))
    if "GAT_ELU_COMBINE" not in dve_ops._SUB_OPCODE_FOR_NAME:
        # out = relu(t) + min(E, 1) - 1  with t=Src0, E=Src1(=exp(t))
        defs.append(DveOp(
            "GAT_ELU_COMBINE",
            Spec(body=relu(Src0) + minn(Src1, One) - One,
                 reference=lambda in0, in1, s0, s1, imm2:
                     np.maximum(in0, 0) + np.minimum(in1, 1.0) - 1.0),
            subdim=False, uops_sha={}))
    for op in defs:
        for ver in ("v3", "v4"):
            tmp = DveOpSpec(name=op.name, opcode=0,
                            uops=lower(op.spec, ver=ver), rd1_en=True)
            op.uops_sha[ver] = tmp.sha(ver)
        dve_ops.OPS.append(op)
        dve_ops.CUSTOM_DVE_SPECS[op.name] = op.spec
        dve_ops._SUB_OPCODE_FOR_NAME[op.name] = (
            dve_ops._CUSTOM_DVE_ROW_BASE + len(dve_ops.OPS) - 1)
    ops = {op.name: op for op in dve_ops.OPS}
    return ops["GAT_MASK_LRELU"], ops["GAT_ELU_COMBINE"]


def build(nc, reps=1):
    op_mask_lrelu, op_elu = _register_ops()

    # ---- I/O ----
    xTs_in = nc.dram_tensor("xTs_in", [FIN, IB], F32, kind="ExternalInput")
    xTsb_in = nc.dram_tensor("xTsb_in", [FIN, IB], BF16, kind="ExternalInput")
    maskT_in = nc.dram_tensor("maskT_in", [N, IB], BF16, kind="ExternalInput")
    Wall_in = nc.dram_tensor("Wall_in", [FIN, FIN], BF16, kind="ExternalInput")
    W12_in = nc.dram_tensor("W12_in", [FIN, 2 * H], F32, kind="ExternalInput")
    Wo_in = nc.dram_tensor("Wo_in", [FIN, FH + 2], F32, kind="ExternalInput")
    wgt_in = nc.dram_tensor("wgt_in", [FH, FH], F32, kind="ExternalInput")
    p1T_in = nc.dram_tensor("p1T_in", [N, PB], BF16, kind="ExternalInput")
    p2T_in = nc.dram_tensor("p2T_in", [N, PB], BF16, kind="ExternalInput")
    scores_out = nc.dram_tensor("scores_out", [1, PB], F32, kind="ExternalOutput")

    groups = [list(range(NC))]

    with tile.TileContext(nc) as tc, ExitStack() as octx:
      for rep in range(reps):
        R = f"_r{rep}"
        ctx = ExitStack()
        octx.enter_context(ctx)
        tiny = ctx.enter_context(tc.tile_pool(name="tiny" + R, bufs=1))
        xcp = ctx.enter_context(tc.tile_pool(name="xcp" + R, bufs=1))
        h2pool = ctx.enter_context(tc.tile_pool(name="h2pool" + R, bufs=1))
        npool = ctx.enter_context(tc.tile_pool(name="npool" + R, bufs=2))
        ppool = ctx.enter_context(tc.tile_pool(name="ppool" + R, bufs=1))
        dram = ctx.enter_context(tc.tile_pool(name="dram" + R, bufs=1, space="DRAM"))
        ps_small = ctx.enter_context(tc.tile_pool(name="ps_small" + R, bufs=2, space="PSUM"))
        ps_h_pool = ctx.enter_context(tc.tile_pool(name="ps_h" + R, bufs=2, space="PSUM"))
        ps_hp_pool = ctx.enter_context(tc.tile_pool(name="ps_hp" + R, bufs=4, space="PSUM"))
        ctx_att1 = ctx.enter_context(ExitStack())
        maskp = ctx_att1.enter_context(tc.tile_pool(name="maskp" + R, bufs=1))
        ztp = ctx_att1.enter_context(tc.tile_pool(name="ztp" + R, bufs=2))
        ep = ctx_att1.enter_context(tc.tile_pool(name="ep" + R, bufs=4))
        ctx_prep = ctx.enter_context(ExitStack())
        fpool = ctx_prep.enter_context(tc.tile_pool(name="fpool" + R, bufs=1))
        hpool = ctx_prep.enter_context(tc.tile_pool(name="hpool" + R, bufs=1))
        ctx_bc = ctx.enter_context(ExitStack())
        cst = ctx_bc.enter_context(tc.tile_pool(name="cst" + R, bufs=1))

        # ---- constant loads ----
        # Two DGE rings: nc.sync (HWDGE/SP) carries the latency-critical small
        # transfers in need-order; nc.gpsimd (SWDGE) carries bulk prefetches so
        # they can't FIFO-block the critical path.
        xTs = []
        xTsb = []
        Wall = []
        W12 = []
        Wo = []
        xTs_g = cst.tile([128, KB, IB], F32, name="xTs_g")
        nc.sync.dma_start(xTs_g[:], xTs_in[:].rearrange("(k p) c -> p k c", p=128))
        xTs = [xTs_g[:, kb, :] for kb in range(KB)]
        W12_g = cst.tile([128, KB, 2 * H], F32, name="W12_g")
        nc.sync.dma_start(W12_g[:], W12_in[:].rearrange("(k p) c -> p k c", p=128))
        W12 = [W12_g[:, kb, :] for kb in range(KB)]
        xTsb_g = cst.tile([128, KB, IB], BF16, name="xTsb_g")
        nc.gpsimd.dma_start(xTsb_g[:], xTsb_in[:].rearrange("(k p) c -> p k c", p=128))
        xTsb = [xTsb_g[:, kb, :] for kb in range(KB)]
        Wall_g = cst.tile([128, KB, FIN], BF16, name="Wall_g")
        nc.gpsimd.dma_start(Wall_g[:], Wall_in[:].rearrange("(k p) c -> p k c", p=128))
        Wall = [Wall_g[:, kb, :] for kb in range(KB)]
        # mask tiles (stay resident through both attention layers); first two
        # groups on the fast ring, the rest later (consumed mid-attention)
        GBM = 4
        mback = []
        for g4 in range(NJ // GBM):
            m = maskp.tile([128, GBM, IB], BF16, name=f"maskT{g4}")
            if g4 < 2:
                nc.sync.dma_start(
                    m[:], maskT_in[g4 * GBM * 128:(g4 + 1) * GBM * 128, :]
                    .rearrange("(g p) c -> p g c", p=128))
            mback.append(m)
        maskT = [mback[jb // GBM][:, jb % GBM, :] for jb in range(NJ)]
        ones8 = tiny.tile([128, H], BF16)
        nc.gpsimd.memset(ones8[:], 1.0)
        ones64 = tiny.tile([FH, 1], F32)
        nc.gpsimd.memset(ones64[:], 1.0)
        ident = tiny.tile([128, 128], F32)
        make_identity(nc, ident[:])

        # ---- stage B: f-pass (fp32, exact): F1 (free-dim) + F2 (natural) ----
        ps_ft = ps_small.tile([2 * H, IB], F32, tag="pss")
        for kb in range(KB):
            nc.tensor.matmul(ps_ft[:], W12[kb][:], xTs[kb][:],
                             start=(kb == 0), stop=(kb == KB - 1))
        FTsb = fpool.tile([2 * H, IB], F32)
        nc.scalar.copy(FTsb[:], ps_ft[:])
        ft_d = dram.tile([2 * H, IB], F32)
        nc.sync.dma_start(ft_d[:], FTsb[:])

        F2loc_sb = []
        for s in range(SUB):
            ps_f2 = ps_small.tile([128, H], F32, tag="pss")
            for kb in range(KB):
                nc.tensor.matmul(ps_f2[:], xTs[kb][:, s * 128:(s + 1) * 128],
                                 W12[kb][:, H:2 * H],
                                 start=(kb == 0), stop=(kb == KB - 1))
            t = fpool.tile([128, H], F32, name=f"F2loc{s}")
            nc.scalar.copy(t[:], ps_f2[:])
            F2loc_sb.append(t)

        # early tiny AllGather of f2 (fp32) so z/exp never waits on the h AG
        f2loc_d = dram.tile([IB, H], F32)
        f2g_d = dram.tile([N, H], F32, addr_space="Shared")
        for s in range(SUB):
            nc.sync.dma_start(f2loc_d[s * 128:(s + 1) * 128, :], F2loc_sb[s][:])
        if SIM_NOCOLL:
            nc.sync.dma_start(f2g_d[0:IB, :], f2loc_d[:])
        else:
            nc.gpsimd.collective_compute(
                "AllGather", mybir.AluOpType.bypass, replica_groups=groups,
                ins=[f2loc_d[:].opt()], outs=[f2g_d[:].opt()])
        f2sb = fpool.tile([128, NJ, H], F32)
        nc.sync.dma_start(
            f2sb[:], f2g_d[:].rearrange("(g p) c -> p g c", p=128))

        # f1 broadcast tiles per head (row bounced to partition 0 via DRAM)
        f1b = []
        for h in range(H):
            row = fpool.tile([1, IB], F32, name=f"f1row{h}")
            nc.sync.dma_start(row[:], ft_d[h:h + 1, :])
            t = fpool.tile([128, IB], F32, name=f"f1b{h}")
            nc.gpsimd.partition_broadcast(t[:], row[:])
            f1b.append(t)

        # remaining mask groups (needed from ~chunk 2 onward)
        for g4 in range(2, NJ // GBM):
            nc.sync.dma_start(
                mback[g4][:], maskT_in[g4 * GBM * 128:(g4 + 1) * GBM * 128, :]
                .rearrange("(g p) c -> p g c", p=128))

        # ---- stage C: local h (bf16) -> haug layout -> gather ----
        hloc_d = dram.tile([IB, HCOLS], BF16)
        for s in range(SUB):
            ps_h = ps_h_pool.tile([128, FIN], F32, tag="ph")
            for kb in range(KB):
                nc.tensor.matmul(ps_h[:], xTsb[kb][:, s * 128:(s + 1) * 128],
                                 Wall[kb][:],
                                 start=(kb == 0), stop=(kb == KB - 1))
            hsb = fpool.tile([128, HCOLS], BF16, name=f"hloc{s}")
            hsb3 = hsb[:].rearrange("p (h f) -> p h f", h=H)
            nc.scalar.copy(hsb3[:, :, 0:FH],
                           ps_h[:].rearrange("p (h f) -> p h f", h=H))
            nc.scalar.copy(hsb3[:, :, FH], ones8[:])
            nc.gpsimd.dma_start(hloc_d[s * 128:(s + 1) * 128, :], hsb[:])

        hg_d = dram.tile([N, HCOLS], BF16, addr_space="Shared")
        if SIM_NOCOLL:
            nc.gpsimd.dma_start(hg_d[0:IB, :], hloc_d[:])
        else:
            nc.gpsimd.collective_compute(
                "AllGather", mybir.AluOpType.bypass, replica_groups=groups,
                ins=[hloc_d[:].opt()], outs=[hg_d[:].opt()])

        # DMA gathered tensor back, 8 j-blocks per transfer (SWDGE ring; the
        # first attention chunks only need DVE/ACT inputs, not haug)
        GB = 8
        hback = []
        for g4 in range(NJ // GB):
            ht = hpool.tile([128, GB, HCOLS], BF16, name=f"hback{g4}")
            nc.gpsimd.dma_start(
                ht[:], hg_d[g4 * GB * 128:(g4 + 1) * GB * 128, :].rearrange(
                    "(g p) c -> p g c", p=128))
            hback.append(ht)
        haug_r = [hback[jb // GB][:, jb % GB, :] for jb in range(NJ)]

        ctx_bc.close()

        # ---- stage D: layer-1 attention, per head ----
        # per-head output tiles on partitions 0..63 so the elu op writes them
        # directly (no partition-moving DMA); layer-2 matmuls contract per-head
        xcT8 = [xcp.tile([FH, IB], F32, name=f"xcT8_{h}") for h in range(H)]

        def attention(head, f2col_of, f1b_t, haug_of, out_t):
            """One attention unit: out_t[:] = elu-ish(att @ h)."""
            ps_hp = ps_hp_pool.tile([FH + 1, IB], F32, tag="hp",
                                    name=f"ps_hp{head}")
            for c in range(NCH):
                zt = ztp.tile([128, CH, IB], F32, tag="zt", name=f"zt{head}_{c}")
                for g in range(CH):
                    jb = c * CH + g
                    nc.vector._custom_dve(
                        op_mask_lrelu, out=zt[:, g, :], in0=f1b_t[:],
                        in1=maskT[jb], s0=f2col_of(jb), s1=ALPHA)
                et = ep.tile([128, CH, IB], BF16, tag="et", name=f"et{head}_{c}")
                nc.scalar.activation(et[:], zt[:], AF.Exp)
                for g in range(CH):
                    jb = c * CH + g
                    nc.tensor.matmul(
                        ps_hp[:], haug_of(jb),
                        et[:, g, :], start=(jb == 0), stop=(jb == NJ - 1))
            # normalize + elu
            # rowsum lives at PSUM partition FH; copy within-partition to
            # SBUF, then DMA (the only partition-moving engine) to partition 0
            rs64 = npool.tile([128, IB], F32, bufs=1, tag="rs64", name=f"rs64_{head}")
            nc.scalar.copy(rs64[FH:FH + 1, :], ps_hp[FH:FH + 1, :])
            rsum = npool.tile([1, IB], F32, bufs=1, tag="rsum", name=f"rsum{head}")
            nc.sync.dma_start(rsum[:], rs64[FH:FH + 1, :])
            rrow = npool.tile([1, IB], F32, bufs=1, tag="rrow", name=f"rrow{head}")
            nc.vector.reciprocal_approx_fast(rrow[:], rsum[:])
            rb = npool.tile([FH, IB], F32, tag="rb", name=f"rb{head}")
            nc.gpsimd.partition_broadcast(rb[:], rrow[:])
            hp_sb = npool.tile([FH, IB], F32, tag="hps", name=f"hps{head}")
            nc.scalar.copy(hp_sb[:], ps_hp[0:FH, :])
            t_n = npool.tile([FH, IB], F32, tag="tn", name=f"tn{head}")
            nc.gpsimd.tensor_mul(t_n[:], hp_sb[:], rb[:])
            e_n = npool.tile([FH, IB], F32, tag="en", name=f"en{head}")
            nc.scalar.activation(e_n[:], t_n[:], AF.Exp)
            nc.vector._custom_dve(op_elu, out=out_t[:], in0=t_n[:], in1=e_n[:])

        p1g = []
        p2g = []
        GP = 8
        for head in range(H):
            attention(head,
                      lambda jb, h=head: f2sb[:, jb, h:h + 1],
                      f1b[head],
                      lambda jb, h=head: haug_r[jb][:, h * HC:h * HC + HC],
                      xcT8[head])
            if head == 0:
                # bulk prefetch for the tail stages, issued once the attention
                # pipeline is running (DMA + Pool are idle from here on)
                Wo_h = tiny.tile([64, H, FH + 2], F32, name="Wo_h")
                nc.gpsimd.dma_start(
                    Wo_h[:], Wo_in[:].rearrange("(h q) c -> q h c", q=64))
                wgt = tiny.tile([FH, FH], F32)
                nc.gpsimd.dma_start(wgt[:], wgt_in[:])
            elif head in (1, 2) and not p2g:
                for g4 in range(NJ // GP):
                    tp1 = ppool.tile([128, GP, PB], BF16, name=f"p1_{g4}")
                    nc.gpsimd.dma_start(
                        tp1[:], p1T_in[g4 * GP * 128:(g4 + 1) * GP * 128, :]
                        .rearrange("(g p) c -> p g c", p=128))
                    p1g.append(tp1)
                    tp2 = ppool.tile([128, GP, PB], BF16, name=f"p2_{g4}")
                    nc.gpsimd.dma_start(
                        tp2[:], p2T_in[g4 * GP * 128:(g4 + 1) * GP * 128, :]
                        .rearrange("(g p) c -> p g c", p=128))
                    p2g.append(tp2)
        p1r = [p1g[jb // GP][:, jb % GP, :] for jb in range(NJ)]
        p2r = [p2g[jb // GP][:, jb % GP, :] for jb in range(NJ)]

        ctx_prep.close()
        xopool = ctx.enter_context(tc.tile_pool(name="xopool" + R, bufs=1))
        epool = ctx.enter_context(tc.tile_pool(name="epool" + R, bufs=1))

        # ---- stage E: layer-2 h (local rows, fp32 in / bf16 out) -> gather ----
        h2loc_d = dram.tile([IB, FH + 2], BF16)
        for s in range(SUB):
            ps_h2 = ps_small.tile([128, FH + 1], F32, tag="pss")
            for h in range(H):
                nc.tensor.matmul(ps_h2[:], xcT8[h][:, s * 128:(s + 1) * 128],
                                 Wo_h[:, h, 0:FH + 1],
                                 start=(h == 0), stop=(h == H - 1))
            t = h2pool.tile([128, FH + 2], BF16, name=f"h2loc{s}")
            nc.scalar.copy(t[:, 0:FH], ps_h2[:, 0:FH])
            nc.scalar.copy(t[:, FH:FH + 1], ones8[:, 0:1])
            nc.scalar.copy(t[:, FH + 1:FH + 2], ps_h2[:, FH:FH + 1])
            nc.sync.dma_start(h2loc_d[s * 128:(s + 1) * 128, :], t[:])
        # f1_2 as a free-dim row
        ps_f12 = ps_small.tile([1, IB], F32, tag="pss")
        for h in range(H):
            nc.tensor.matmul(ps_f12[:], Wo_h[:, h, FH + 1:FH + 2], xcT8[h][:],
                             start=(h == 0), stop=(h == H - 1))
        f12row = h2pool.tile([1, IB], F32)
        nc.scalar.copy(f12row[:], ps_f12[:])
        f12b = h2pool.tile([128, IB], F32)
        nc.gpsimd.partition_broadcast(f12b[:], f12row[:])

        h2g_d = dram.tile([N, FH + 2], BF16, addr_space="Shared")
        if SIM_NOCOLL:
            nc.sync.dma_start(h2g_d[0:IB, :], h2loc_d[:])
        else:
            nc.gpsimd.collective_compute(
                "AllGather", mybir.AluOpType.bypass, replica_groups=groups,
                ins=[h2loc_d[:].opt()], outs=[h2g_d[:].opt()])

        h2back = h2pool.tile([128, NJ, FH + 2], BF16)
        nc.sync.dma_start(
            h2back[:], h2g_d[:].rearrange("(g p) c -> p g c", p=128))
        h2r = [h2back[:, jb, 0:FH + 1] for jb in range(NJ)]
        # layer-2 f2 column cast to fp32 (per-partition scalar for the z op)
        f22sb = h2pool.tile([128, NJ], F32)
        nc.scalar.copy(f22sb[:], h2back[:, :, FH + 1])

        # ---- stage F: layer-2 attention (single head) ----
        xoT = h2pool.tile([FH, IB], F32)

        ps_hp2 = ps_hp_pool.tile([FH + 1, IB], F32, tag="hp", name="ps_hp2")
        for c in range(NCH):
            zt = ztp.tile([128, CH, IB], F32, tag="zt", name=f"zt2_{c}")
            for g in range(CH):
                jb = c * CH + g
                nc.vector._custom_dve(
                    op_mask_lrelu, out=zt[:, g, :], in0=f12b[:],
                    in1=maskT[jb], s0=f22sb[:, jb:jb + 1], s1=ALPHA)
            et = ep.tile([128, CH, IB], BF16, tag="et", name=f"et2_{c}")
            nc.scalar.activation(et[:], zt[:], AF.Exp)
            for g in range(CH):
                jb = c * CH + g
                nc.tensor.matmul(ps_hp2[:], h2r[jb], et[:, g, :],
                                 start=(jb == 0), stop=(jb == NJ - 1))
        rs64b = npool.tile([128, IB], F32, bufs=1, tag="rs64", name="rs64b")
        nc.scalar.copy(rs64b[FH:FH + 1, :], ps_hp2[FH:FH + 1, :])
        rsum2 = npool.tile([1, IB], F32, bufs=1, tag="rsum", name="rsum2")
        nc.sync.dma_start(rsum2[:], rs64b[FH:FH + 1, :])
        rrow2 = npool.tile([1, IB], F32, bufs=1, tag="rrow", name="rrow2")
        nc.vector.reciprocal_approx_fast(rrow2[:], rsum2[:])
        rb2 = npool.tile([FH, IB], F32, tag="rb", name="rb2")
        nc.gpsimd.partition_broadcast(rb2[:], rrow2[:])
        hp_sb2 = npool.tile([FH, IB], F32, tag="hps", name="hps2")
        nc.scalar.copy(hp_sb2[:], ps_hp2[0:FH, :])
        t_n2 = npool.tile([FH, IB], F32, tag="tn", name="tn2")
        nc.gpsimd.tensor_mul(t_n2[:], hp_sb2[:], rb2[:])
        e_n2 = npool.tile([FH, IB], F32, tag="en", name="en2")
        nc.scalar.activation(e_n2[:], t_n2[:], AF.Exp)
        nc.vector._custom_dve(op_elu, out=xoT[:], in0=t_n2[:], in1=e_n2[:])

        # ---- stage G: x_out natural layout (bf16) + gather ----
        xoloc_d = dram.tile([IB, FH], BF16)
        for s in range(SUB):
            ps_tr = ps_small.tile([128, FH], F32, tag="pss")
            nc.tensor.transpose(ps_tr[:], xoT[:, s * 128:(s + 1) * 128], ident[0:FH, 0:FH])
            t = xopool.tile([128, FH], BF16, name=f"xol{s}")
            nc.scalar.copy(t[:], ps_tr[:])
            nc.sync.dma_start(xoloc_d[s * 128:(s + 1) * 128, :], t[:])
        xog_d = dram.tile([N, FH], BF16, addr_space="Shared")
        if SIM_NOCOLL:
            nc.sync.dma_start(xog_d[0:IB, :], xoloc_d[:])
        else:
            nc.gpsimd.collective_compute(
                "AllGather", mybir.AluOpType.bypass, replica_groups=groups,
                ins=[xoloc_d[:].opt()], outs=[xog_d[:].opt()])

        xorg = xopool.tile([128, NJ, FH], BF16)
        nc.sync.dma_start(
            xorg[:], xog_d[:].rearrange("(g p) c -> p g c", p=128))
        xor_ = [xorg[:, jb, :] for jb in range(NJ)]

        # ---- stage H: pair embeddings + scores ----
        ps_e1 = ps_small.tile([FH, PB], F32, tag="pss", name="ps_e1")
        for jb in range(NJ):
            nc.tensor.matmul(ps_e1[:], xor_[jb], p1r[jb],
                             start=(jb == 0), stop=(jb == NJ - 1))
        e1sb = epool.tile([FH, PB], F32)
        nc.scalar.copy(e1sb[:], ps_e1[:])
        ps_e2 = ps_small.tile([FH, PB], F32, tag="pss", name="ps_e2")
        for jb in range(NJ):
            nc.tensor.matmul(ps_e2[:], xor_[jb], p2r[jb],
                             start=(jb == 0), stop=(jb == NJ - 1))
        e2sb = epool.tile([FH, PB], F32)
        nc.scalar.copy(e2sb[:], ps_e2[:])

        ps_g = ps_small.tile([FH, PB], F32, tag="pss", name="ps_g")
        nc.tensor.matmul(ps_g[:], wgt[:], e1sb[:], start=True, stop=True)
        prod = epool.tile([FH, PB], F32)
        nc.vector.tensor_mul(prod[:], ps_g[:], e2sb[:])
        ps_s = ps_small.tile([1, PB], F32, tag="pss", name="ps_s")
        nc.tensor.matmul(ps_s[:], ones64[:], prod[:], start=True, stop=True)
        srow = epool.tile([1, PB], F32)
        nc.scalar.copy(srow[:], ps_s[:])
        nc.sync.dma_start(scores_out[:], srow[:])
        ctx.close()

    return nc


_CACHE = {}


def _get_nc(reps=1):
    key = f"nc{reps}"
    if key not in _CACHE:
        nc = bacc.Bacc(None, target_bir_lowering=False, debug=False, num_devices=NC)
        build(nc, reps=reps)
        nc.compile()
        _CACHE[key] = nc
    return _CACHE[key]


def _bf16(a):
    import ml_dtypes
    return np.ascontiguousarray(np.asarray(a, np.float32).astype(ml_dtypes.bfloat16))


def prep_inputs(x, adj, pair1_map, pair2_map, Wh, a1h, a2h, W_out, a1_out,
                a2_out, weight):
    x = np.ascontiguousarray(np.asarray(x, np.float32))
    adj = np.asarray(adj)
    maskT = np.where(adj > 0, np.float32(0.0), np.float32(MASKVAL)).T  # [j, i]
    maskT = _bf16(maskT)
    xT = np.ascontiguousarray(x.T)                                     # [FIN, N]
    Wall = _bf16(np.ascontiguousarray(
        np.transpose(np.asarray(Wh, np.float64), (1, 0, 2)).reshape(FIN, H * FH)))
    w1 = np.einsum("hkf,hf->kh", np.asarray(Wh, np.float64), np.asarray(a1h, np.float64))
    w2 = np.einsum("hkf,hf->kh", np.asarray(Wh, np.float64), np.asarray(a2h, np.float64))
    W12 = np.concatenate([w1, w2], axis=1).astype(np.float32)          # [FIN, 16]
    w1o = np.asarray(W_out, np.float64) @ np.asarray(a1_out, np.float64)
    w2o = np.asarray(W_out, np.float64) @ np.asarray(a2_out, np.float64)
    Wo = np.concatenate([np.asarray(W_out, np.float64), w2o[:, None],
                         w1o[:, None]], axis=1).astype(np.float32)     # [FIN, 66]
    p1T = _bf16(np.asarray(pair1_map, np.float32).T)                   # [N, NPAIR]
    p2T = _bf16(np.asarray(pair2_map, np.float32).T)
    wgt = np.ascontiguousarray(np.asarray(weight, np.float32))

    in_maps = []
    for c in range(NC):
        i0, i1 = c * IB, (c + 1) * IB
        p0, p1 = c * PB, (c + 1) * PB
        xs = np.ascontiguousarray(xT[:, i0:i1])
        in_maps.append({
            "xTs_in": xs,
            "xTsb_in": _bf16(xs),
            "maskT_in": np.ascontiguousarray(maskT[:, i0:i1]),
            "Wall_in": Wall,
            "W12_in": W12,
            "Wo_in": Wo,
            "wgt_in": wgt,
            "p1T_in": np.ascontiguousarray(p1T[:, p0:p1]),
            "p2T_in": np.ascontiguousarray(p2T[:, p0:p1]),
        })
    return in_maps


def run(inputs, trace=False, **kw):
    nc = _get_nc()
    in_maps = prep_inputs(**inputs)
    res = run_bass_kernel_spmd(nc, in_maps, list(range(NC)), trace=trace, **kw)
    scores = np.concatenate(
        [res.results[c]["scores_out"].reshape(-1) for c in range(NC)])
    return scores.astype(np.float32), res


def kernel(**inputs):
    return run(inputs)[0]


def _make_fn(nc, in_maps):
    import jax
    from jax.sharding import Mesh, PartitionSpec, NamedSharding
    from jax.experimental.shard_map import shard_map
    from concourse import bass2jax
    import concourse.mybir as _mb

    bass2jax.install_neuronx_cc_hook()
    partition_name = nc.partition_id_tensor.name if nc.partition_id_tensor else None
    in_names, out_names, out_avals, zero_outs = [], [], [], []
    for alloc in nc.m.functions[0].allocations:
        if not isinstance(alloc, _mb.MemoryLocationSet):
            continue
        name = alloc.memorylocations[0].name
        if alloc.kind == "ExternalInput":
            if name != partition_name:
                in_names.append(name)
        elif alloc.kind == "ExternalOutput":
            shape = list(alloc.tensor_shape)
            npdt = _mb.dt.np(alloc.dtype)
            out_names.append(name)
            out_avals.append(jax.core.ShapedArray(shape, npdt))
            zero_outs.append(np.zeros(shape, npdt))
    n_params = len(in_names)
    n_outs = len(out_names)
    all_in_names = list(in_names) + list(out_names)
    if partition_name is not None:
        all_in_names.append(partition_name)

    def _body(*args):
        operands = list(args)
        if partition_name is not None:
            operands.append(bass2jax.partition_id_tensor())
        outs = bass2jax._bass_exec_p.bind(
            *operands, out_avals=tuple(out_avals), in_names=tuple(all_in_names),
            out_names=tuple(out_names), lowering_input_output_aliases=(),
            sim_require_finite=True, sim_require_nnan=True, nc=nc)
        return tuple(outs)

    devices = jax.devices()[:NC]
    mesh = Mesh(np.asarray(devices), ("core",))
    in_specs = (PartitionSpec("core"),) * (n_params + n_outs)
    out_specs = (PartitionSpec("core"),) * n_outs
    fn = jax.jit(shard_map(_body, mesh=mesh, in_specs=in_specs,
                           out_specs=out_specs, check_rep=False),
                 keep_unused=True)
    concat_in = [
        np.concatenate([np.asarray(in_maps[c][nm]) for c in range(NC)], axis=0)
        for nm in in_names]
    concat_zeros = [np.zeros((NC * z.shape[0], *z.shape[1:]), z.dtype)
                    for z in zero_outs]
    sh = NamedSharding(mesh, PartitionSpec("core"))
    dev_in = [jax.device_put(a, sh) for a in concat_in]
    dev_zero = [jax.device_put(a, sh) for a in concat_zeros]
    return fn, dev_in, dev_zero


def bench(inputs, iters=6, kreps=9):
    """Device time per kernel pass, via the in-NEFF replication slope."""
    import time
    import jax
    in_maps = prep_inputs(**inputs)
    fns = {}
    for reps in (1, kreps):
        nc = _get_nc(reps=reps)
        fn, dev_in, dev_zero = _make_fn(nc, in_maps)
        jax.block_until_ready(fn(*dev_in, *dev_zero))  # warm/compile
        fns[reps] = (fn, dev_in, dev_zero)

    def once(reps):
        fn, dev_in, dev_zero = fns[reps]
        t0 = time.perf_counter()
        jax.block_until_ready(fn(*dev_in, *dev_zero))
        return time.perf_counter() - t0

    t1s, tks, diffs = [], [], []
    for _ in range(3 * iters):
        a = once(1)
        b = once(kreps)
        c = once(1)
        t1s += [a, c]
        tks.append(b)
        diffs.append(b - (a + c) / 2)
    diffs.sort()
    n = len(diffs)
    med = diffs[n // 2]
    scale = 1e9 / (kreps - 1)
    out = {
        "t1_ns": min(t1s) * 1e9,
        f"t{kreps}_ns": min(tks) * 1e9,
        "p25_ns": diffs[n // 4] * scale,
        "p75_ns": diffs[(3 * n) // 4] * scale,
        "minmin_ns": (min(tks) - min(t1s)) * scale,
        "pooled_med_ns": med * scale,
        "per_exec_ns": max(med * scale, 0.0),
    }
    return out


if __name__ == "__main__":
    rng = np.random.default_rng(0)
    ins = dict(
        x=rng.standard_normal((N, FIN), dtype=np.float32),
        adj=(rng.random((N, N)) < 0.5).astype(np.int32),
        pair1_map=rng.standard_normal((NPAIR, N), dtype=np.float32),
        pair2_map=rng.standard_normal((NPAIR, N), dtype=np.float32),
        Wh=rng.standard_normal((H, FIN, FH), dtype=np.float32) * 0.1,
        a1h=rng.standard_normal((H, FH), dtype=np.float32) * 0.3,
        a2h=rng.standard_normal((H, FH), dtype=np.float32) * 0.3,
        W_out=rng.standard_normal((FIN, FH), dtype=np.float32) * 0.1,
        a1_out=rng.standard_normal((FH,), dtype=np.float32) * 0.3,
        a2_out=rng.standard_normal((FH,), dtype=np.float32) * 0.3,
        weight=rng.standard_normal((FH, FH), dtype=np.float32) * 0.1,
    )
    out = kernel(**ins)
    print("scores:", out.shape, out[:8])
